# revision 4
# baseline (speedup 1.0000x reference)
"""Trainium2 Bass kernel for nn_Attention_17454747091547 (DErf formulation,
2-tile-paired elementwise ops).

See kernel.py (v2) docstring for the math. v3 processes tiles in pairs:
PE matmuls, DErf-B and transposes stay per-tile (PSUM limits), while all
DVE/GpSimd elementwise ops and the small ACT ops work on [P, 2, ...]
pair tiles, halving per-instruction init/launch overheads.
"""

import numpy as np
import ml_dtypes

import concourse.bass as bass
import concourse.tile as tile
from concourse import bacc
from concourse import mybir
from concourse.bass_utils import run_bass_kernel_spmd

SEG = [0, 1, 1, 1, 1, 1, 2, 2, 2, 3, 4, 4, 4, 4, 4, 4]
N_CORES = 8
B_TOTAL = 262144
B_SHARD = B_TOTAL // N_CORES  # 32768
P = 128
NTILES = B_SHARD // P  # 256
NPAIRS = NTILES // 2
S = 16
D = 6
H = 2
HD = 3
F = S * D  # 96
NB = HD * H * S * S   # 1536
NK = HD * H * S       # 96
NV = H * 4 * S        # 128
NKV = NK + NV         # 224
NW = NB + NKV         # 1760
SIGMA = float(1.0 / np.sqrt(2.0 * np.sqrt(np.float32(HD))))

_nc_cache = {}


def _build_graph():
    nc = bacc.Bacc()
    f32 = mybir.dt.float32
    bf16 = mybir.dt.bfloat16
    add = mybir.AluOpType.add
    mult = mybir.AluOpType.mult
    DErf = mybir.ActivationFunctionType.Derivative_Erf
    Copy = mybir.ActivationFunctionType.Copy

    xt_ext = nc.declare_dram_parameter("xt", [F + 1, B_SHARD], bf16, isOutput=False)
    wf_ext = nc.declare_dram_parameter("wf", [F + 1, NW], bf16, isOutput=False)
    wo_ext = nc.declare_dram_parameter("wo", [F, F], bf16, isOutput=False)
    bo_ext = nc.declare_dram_parameter("bo", [F], f32, isOutput=False)
    id_ext = nc.declare_dram_parameter("ident", [P, P], bf16, isOutput=False)
    out_ext = nc.declare_dram_parameter("out", [F, B_SHARD], bf16, isOutput=True)

    with tile.TileContext(nc) as tc:
        with (
            tc.tile_pool(name="const", bufs=1) as const,
            tc.tile_pool(name="sbE", bufs=3) as sbE,
            tc.tile_pool(name="sbwork", bufs=2) as sbwork,
            tc.tile_pool(name="sbsmall", bufs=3) as sbsmall,
            tc.tile_pool(name="sbctxT", bufs=3) as sbctxT,
            tc.tile_pool(name="psB", bufs=1, space="PSUM") as psBp,
            tc.tile_pool(name="psKV", bufs=2, space="PSUM") as psKVp,
            tc.tile_pool(name="psT", bufs=1, space="PSUM") as psTp,
            tc.tile_pool(name="psO", bufs=1, space="PSUM") as psOp,
        ):
            # --- setup: stage PE-read constants through DVE (sync-wait limit)
            wf_dma = const.tile([F + 1, NW], bf16, name="wf_dma")
            nc.sync.dma_start(out=wf_dma, in_=wf_ext[:])
            wf_sb = const.tile([F + 1, NW], bf16, name="wf_sb")
            nc.vector.tensor_copy(wf_sb[:], wf_dma[:])
            wo_dma = const.tile([F, F], bf16, name="wo_dma")
            nc.sync.dma_start(out=wo_dma, in_=wo_ext[:])
            wo_sb = const.tile([F, F], bf16, name="wo_sb")
            nc.vector.tensor_copy(wo_sb[:], wo_dma[:])
            id_dma = const.tile([P, P], bf16)
            nc.sync.dma_start(out=id_dma, in_=id_ext[:])
            I128b = const.tile([P, P], bf16)
            nc.vector.tensor_copy(I128b[:], id_dma[:])
            bo_dma = const.tile([F, 1], f32)
            nc.sync.dma_start(out=bo_dma, in_=bo_ext[:].unsqueeze(1))
            bo_sb = const.tile([F, 1], f32)
            nc.vector.tensor_copy(bo_sb[:], bo_dma[:])

            # dummy DErf: loads the erf_derivative ACT table during setup
            act_warm = const.tile([1, 1], f32)
            nc.scalar.activation(act_warm, bo_sb[0:1, 0:1], DErf)

            # dummy matmul: PE observes the DVE setup tick
            psDummy = psTp.tile([1, 1], f32, tag="t")
            nc.tensor.matmul(psDummy, lhsT=I128b[0:1, 0:1],
                             rhs=I128b[0:1, 0:1], start=True, stop=True)

            # x chunks (finer leading chunks -> earlier tile-0 start)
            chunk_tiles = [1, 1, 2, 4] + [8] * ((NTILES - 8) // 8)
            assert sum(chunk_tiles) == NTILES
            xchunk_of_tile = []
            xcol_of_tile = []
            xc = []
            t0 = 0
            for c, nt in enumerate(chunk_tiles):
                xtile = const.tile([F + 1, nt * P], bf16, tag=f"xc{c}",
                                   name=f"xc_{c}")
                nc.sync.dma_start(
                    out=xtile,
                    in_=xt_ext[:, t0 * P:(t0 + nt) * P])
                xc.append(xtile)
                for j in range(nt):
                    xchunk_of_tile.append(c)
                    xcol_of_tile.append(j * P)
                t0 += nt

            # transposed output accumulators (pair-aligned chunks)
            out_chunk_tiles = [32] * 7 + [16, 8, 4, 2, 2]
            assert sum(out_chunk_tiles) == NTILES
            outc = []
            ochunk_of_tile = []
            ocol_of_tile = []
            ostart_col = []
            t0 = 0
            for c, nt in enumerate(out_chunk_tiles):
                otile = const.tile([F, nt * P], bf16, tag=f"oc{c}",
                                   name=f"outc_{c}")
                outc.append(otile)
                ostart_col.append(t0 * P)
                for j in range(nt):
                    ochunk_of_tile.append(c)
                    ocol_of_tile.append(j * P)
                t0 += nt

            for ip in range(NPAIRS):
                ts = (2 * ip, 2 * ip + 1)
                xsl = [xc[xchunk_of_tile[it]][
                    :, xcol_of_tile[it]:xcol_of_tile[it] + P] for it in ts]

                # per-pair KV matmuls into one bank; per-tile B matmuls into
                # the single-buffered 3-bank psB, DErf'd into pair halves
                psKV = psKVp.tile([P, 2, NKV], f32, tag="kv")
                E3 = sbE.tile([P, 2, HD, H * S * S], bf16, tag="E3")
                for t in (0, 1):
                    nc.tensor.matmul(psKV[:, t], lhsT=xsl[t],
                                     rhs=wf_sb[:, NB:NW],
                                     start=True, stop=True)
                    psB = psBp.tile([P, NB], f32, tag="b")
                    for c0 in range(0, NB, 512):
                        nc.tensor.matmul(psB[:, c0:c0 + 512], lhsT=xsl[t],
                                         rhs=wf_sb[:, c0:c0 + 512],
                                         start=True, stop=True)
                    nc.scalar.activation(
                        E3[:, t].rearrange("p a b -> p (a b)"), psB[:], DErf)

                # K~ Gaussians + V eviction (pair-strided ACT ops)
                K3 = sbsmall.tile([P, 2, HD, H * S], bf16, tag="K3")
                nc.scalar.activation(
                    K3[:].rearrange("p t a b -> p t (a b)"),
                    psKV[:, :, 0:NK], DErf)
                V4 = sbsmall.tile([P, 2, H, 4, S], bf16, tag="V4")
                nc.scalar.activation(
                    V4[:].rearrange("p t a b c -> p t (a b c)"),
                    psKV[:, :, NK:NKV], Copy)

                # E = prod_d E3[d]
                E01 = sbE.tile([P, 2, H * S * S], bf16, tag="E01")
                nc.vector.tensor_tensor(E01[:], E3[:, :, 0], E3[:, :, 1],
                                        op=mult)
                E = sbE.tile([P, 2, H, S, S], bf16, tag="E")
                nc.vector.tensor_tensor(
                    E[:].rearrange("p t a b c -> p t (a b c)"), E01[:],
                    E3[:, :, 2], op=mult)

                # g = 1 / prod_d K3[d]
                gi01 = sbsmall.tile([P, 2, H * S], bf16, tag="gi01")
                nc.gpsimd.tensor_tensor(gi01[:], K3[:, :, 0], K3[:, :, 1],
                                        op=mult)
                gi = sbsmall.tile([P, 2, H * S], f32, tag="gi")
                nc.gpsimd.tensor_tensor(gi[:], gi01[:], K3[:, :, 2], op=mult)
                g = sbsmall.tile([P, 2, H, S], f32, tag="g")
                nc.vector.reciprocal(
                    g[:].rearrange("p t a b -> p (t a b)"),
                    gi[:].rearrange("p t a -> p (t a)"))

                # Vg[h,d4,k] = V[h,d4,k] * g[h,k]
                Vg = sbsmall.tile([P, 2, H, 4, S], bf16, tag="Vg")
                nc.gpsimd.tensor_tensor(
                    Vg[:], V4[:],
                    g[:].unsqueeze(3).broadcast_to([P, 2, H, 4, S]), op=mult)

                # ctx+denom: prod2[h,q,d4,k] = E[h,q,k]*Vg[h,d4,k]
                prod2 = sbwork.tile([P, 2, H, S, 4, S], bf16, tag="prod2")
                nc.vector.tensor_tensor(
                    prod2[:],
                    E[:].unsqueeze(4).broadcast_to([P, 2, H, S, 4, S]),
                    Vg[:].unsqueeze(3).broadcast_to([P, 2, H, S, 4, S]),
                    op=mult)
                t8 = sbwork.tile([P, 2, H, S, 4, 8], bf16, tag="t8")
                nc.vector.tensor_tensor(
                    t8[:], prod2[:, :, :, :, :, 0:8],
                    prod2[:, :, :, :, :, 8:16], op=add)
                t4 = sbwork.tile([P, 2, H, S, 4, 4], bf16, tag="t4")
                nc.vector.tensor_tensor(
                    t4[:, :, 0], t8[:, :, 0, :, :, 0:4],
                    t8[:, :, 0, :, :, 4:8], op=add)
                nc.gpsimd.tensor_tensor(
                    t4[:, :, 1], t8[:, :, 1, :, :, 0:4],
                    t8[:, :, 1, :, :, 4:8], op=add)
                t2 = sbwork.tile([P, 2, H, S, 4, 2], bf16, tag="t2")
                nc.gpsimd.tensor_tensor(
                    t2[:], t4[:, :, :, :, :, 0:2], t4[:, :, :, :, :, 2:4],
                    op=add)
                ctxu4 = sbwork.tile([P, 2, H, S, 4], f32, tag="ctxu4")
                nc.gpsimd.tensor_tensor(
                    ctxu4[:], t2[:, :, :, :, :, 0], t2[:, :, :, :, :, 1],
                    op=add)

                # normalize
                rden = sbsmall.tile([P, 2, H, S], f32, tag="rden")
                nc.vector.reciprocal(
                    rden[:].rearrange("p t a b -> p (t a b)"),
                    ctxu4[:, :, :, :, 3].rearrange("p t a b -> p (t a b)"))
                ctx_b = sbsmall.tile([P, 2, H, S, HD], bf16, tag="ctxb")
                nc.gpsimd.tensor_tensor(
                    ctx_b[:],
                    ctxu4[:, :, :, :, 0:HD],
                    rden[:].unsqueeze(4).broadcast_to([P, 2, H, S, HD]),
                    op=mult)

                # out-proj (transposed): per-tile transposes, paired matmul.
                # ctx_b is h-major; wo is row-permuted host-side to match.
                psC = psTp.tile([F, 2 * P], bf16, tag="t")
                for t in (0, 1):
                    nc.tensor.transpose(
                        psC[:, t * P:(t + 1) * P],
                        ctx_b[:, t].rearrange("p h s d -> p (h s d)"),
                        I128b[:])
                ctxT = sbctxT.tile([F, 2 * P], bf16)
                nc.scalar.activation(ctxT[:], psC[:], Copy)
                psOutT = psOp.tile([F, 2 * P], f32)
                nc.tensor.matmul(psOutT, lhsT=wo_sb[:], rhs=ctxT[:],
                                 start=True, stop=True)
                oc_idx = ochunk_of_tile[ts[0]]
                oc_off = ocol_of_tile[ts[0]]
                nc.scalar.activation(
                    outc[oc_idx][:, oc_off:oc_off + 2 * P], psOutT[:],
                    mybir.ActivationFunctionType.Identity,
                    bias=bo_sb[:], scale=1.0)

            for c, nt in enumerate(out_chunk_tiles):
                nc.sync.dma_start(
                    out=out_ext[:, ostart_col[c]:ostart_col[c] + nt * P],
                    in_=outc[c][:])

    return nc


def get_graph():
    if "nc" not in _nc_cache:
        nc = _build_graph()
        nc.finalize()
        _nc_cache["nc"] = nc
    return _nc_cache["nc"]


def _build_wfull(Wq, bq, Wk, bk, Wv, bv):
    """[97, 1760] f32: B-block [d][h][q][k] | K~-block [d][h][k] | V [h][d4][k]."""
    seg = np.asarray(SEG)

    def proj_cols(W, b):
        cols = np.zeros((S, H, HD, F + 1), np.float32)
        for pos in range(S):
            Wp = W[seg[pos]]
            for h in range(H):
                for d in range(HD):
                    cols[pos, h, d, pos * D:(pos + 1) * D] = Wp[h * HD + d]
                    cols[pos, h, d, F] = b[seg[pos]][h * HD + d]
        return cols

    cq = proj_cols(Wq, bq)
    ck = proj_cols(Wk, bk)
    cv = proj_cols(Wv, bv)

    W_B = np.zeros((F + 1, NB), np.float32)
    i = 0
    for d in range(HD):
        for h in range(H):
            for q in range(S):
                for k in range(S):
                    W_B[:, i] = SIGMA * (cq[q, h, d] - ck[k, h, d])
                    i += 1
    W_K = np.zeros((F + 1, NK), np.float32)
    i = 0
    for d in range(HD):
        for h in range(H):
            for k in range(S):
                W_K[:, i] = SIGMA * ck[k, h, d]
                i += 1
    W_V = np.zeros((F + 1, NV), np.float32)
    i = 0
    for h in range(H):
        for d4 in range(4):
            for k in range(S):
                if d4 < HD:
                    W_V[:, i] = cv[k, h, d4]
                else:
                    W_V[F, i] = 1.0
                i += 1
    return np.concatenate([W_B, W_K, W_V], axis=1)


def prepare_in_maps(x, Wq, bq, Wk, bk, Wv, bv, Wo, bo):
    bf16 = ml_dtypes.bfloat16
    wf = _build_wfull(Wq, bq, Wk, bk, Wv, bv).astype(bf16)
    wo_full = np.zeros((F, F), np.float32)
    for s in range(S):
        wo_full[s * D:(s + 1) * D, s * D:(s + 1) * D] = Wo.T
    # ctxT rows arrive in (h,s,d) order; permute wo rows accordingly
    perm = np.empty(F, np.int64)
    for h in range(H):
        for s in range(S):
            for d in range(HD):
                perm[h * S * HD + s * HD + d] = s * D + h * HD + d
    wo = wo_full[perm, :].astype(bf16)
    bof = np.tile(bo, S).astype(np.float32)
    ident = np.eye(P, dtype=bf16)

    xf = np.asarray(x, np.float32).reshape(B_TOTAL, F).astype(bf16)
    ones = np.ones((1, B_SHARD), dtype=bf16)
    in_maps = []
    for c in range(N_CORES):
        shard = np.concatenate([np.ascontiguousarray(
            xf[c * B_SHARD:(c + 1) * B_SHARD].T), ones], axis=0)  # [97, B]
        in_maps.append({"xt": shard, "wf": wf, "wo": wo,
                        "bo": bof, "ident": ident})
    return in_maps


def kernel(x, Wq, bq, Wk, bk, Wv, bv, Wo, bo):
    nc = get_graph()
    in_maps = prepare_in_maps(x, Wq, bq, Wk, bk, Wv, bv, Wo, bo)
    res = run_bass_kernel_spmd(nc, in_maps, core_ids=list(range(N_CORES)))
    outs = [np.asarray(res.results[c]["out"]).astype(np.float32).T
            for c in range(N_CORES)]  # each [32768, 96]
    out = np.concatenate(outs, axis=0)
    return np.ascontiguousarray(out.reshape(B_TOTAL, S, D))
